# revision 40
# baseline (speedup 1.0000x reference)
"""Weighted-BCE loss kernel for Trainium2 (8 NeuronCores, SPMD data-parallel).

Reference math (torch-style BCELoss with class-balancing weights):
    n   = len(x), s = sum(gt)
    w0  = n / (2*(n-s)),  w1 = n / (2*s)
    L1  = max(log(x),     -100)
    L0  = max(log1p(-x),  -100)
    loss = mean( where(gt==0, w0, w1) * -(gt*L1 + (1-gt)*L0) )

The weights depend only on the GLOBAL positive count s, so the loss
decomposes into 4 global sums computed shard-locally:
    A = sum(gt * L1),  B = sum(gt * L0u),  C = sum(L0u),  s = sum(gt)
    loss = -( A/(2s) + (C-B)/(2(n-s)) )
L0u is UNclamped log(1-x): x is fp32 in [0,1), so 1-x >= 2^-25 and
log(1-x) >= -17.4 — the -100 clamp can never fire on the L0 branch.
The L1 clamp IS needed (x == 0 -> ACT Ln gives -inf, measured) and rides
for free inside the DVE's fused scalar_tensor_tensor op.

Engine split per 1/8 shard (2M elements as [128 partitions, 16384 free]):
  - gt is narrowed to bf16 on the host (0/1 — exact): 2/3 the DMA bytes,
    PE-compatible dtype, and 2x-mode DVE operand.
  - ScalarE (ACT) is the critical engine: exactly two Ln passes per tile
    (ACT is 1x rate at every dtype, measured), writing bf16, no accum.
  - VectorE (DVE):
      A += reduce((lnx max -100) * gt)   fused scalar_tensor_tensor
      prodB = gt * ln1                   tensor_tensor, 2x bf16 mode
    A's short cascade (ACT->DVE->out) keeps the kernel tail small.
  - TensorE (PE, otherwise idle) reduces the rest against a fixed
    ones[128,1] stationary at full rate (512 cols / 213ns), each sum
    accumulating into its own PSUM bank across all tiles:
      S += ones.T @ gt_chunk     C += ones.T @ ln1_chunk
      B += ones.T @ prodB_chunk
  - DMA: single sync HWDGE ring; x tiles lead gt tiles by one wave so
    ACT never starves (measured gapless); 8 tapered tile sizes shrink
    both the first-tile ramp and the multi-engine drain at the end.
  - Tail: S bank copied to SBUF early, B+C right after their last
    matmuls; one 8KB outSABC DMA + one tiny outA DMA.
Host gathers the partial rows from all 8 cores and finishes the (tiny)
all-reduce + final scalar arithmetic in float64.
"""

import numpy as np
import ml_dtypes
from contextlib import ExitStack

import concourse.bass as bass
import concourse.bacc as bacc
import concourse.mybir as mybir
import concourse.tile as tile
from concourse.alu_op_type import AluOpType
from concourse.bass_utils import run_bass_kernel_spmd

N_TOTAL = 16777216
N_CORES = 8
PER_CORE = N_TOTAL // N_CORES   # 2097152
P = 128
FD = PER_CORE // P              # 16384 free elements per partition
TILE_SIZES = [2048, 3072, 4096, 4608, 2048, 512]
assert sum(TILE_SIZES) == FD
NT = len(TILE_SIZES)
LOG_CLAMP = -100.0

# Optional instrumentation knobs for a driver script (harness never sets them).
TRACE = False
LAST_RESULTS = None

_NC_CACHE = None


def _build():
    f32 = mybir.dt.float32
    bf16 = mybir.dt.bfloat16
    Ln = mybir.ActivationFunctionType.Ln

    nc = bacc.Bacc("TRN2")
    x_in = nc.declare_dram_parameter("x", [P, FD], f32, isOutput=False)
    g_in = nc.declare_dram_parameter("gt", [P, FD], bf16, isOutput=False)
    outA = nc.declare_dram_parameter("outA", [P, NT], f32, isOutput=True)
    # [S | B | C] partial rows, 512 columns each, packed side by side
    outSBC = nc.declare_dram_parameter("outSBC", [1, 1536], f32, isOutput=True)

    with tile.TileContext(nc) as tc, ExitStack() as ctx:
        xp = ctx.enter_context(tc.tile_pool(name="xp", bufs=4))
        gp = ctx.enter_context(tc.tile_pool(name="gp", bufs=4))
        lp = ctx.enter_context(tc.tile_pool(name="lp", bufs=3))
        prp = ctx.enter_context(tc.tile_pool(name="prp", bufs=3))
        jp = ctx.enter_context(tc.tile_pool(name="jp", bufs=1))
        accp = ctx.enter_context(tc.tile_pool(name="accp", bufs=1))
        pp = ctx.enter_context(tc.psum_pool(name="pp", bufs=1))

        accA = accp.tile([P, NT], f32)
        ones = accp.tile([P, 1], bf16)
        nc.gpsimd.memset(ones[:], 1.0)

        # one psum tile spanning 3 banks: S, B, C (one bank each; a bank
        # is 2KB/partition = 512 f32 and the matmul start-bit resets a
        # whole bank, so each sum owns one)
        psAll = pp.tile([1, 3 * 512], f32)
        psS = psAll[:, 0:512]
        psB = psAll[:, 512:1024]
        psC = psAll[:, 1024:1536]

        ns_total = FD // 512
        done = {"S": 0, "B": 0, "C": 0}

        def reduce_chunks(ps, key, src, tfd):
            for c in range(tfd // 512):
                cs = slice(c * 512, (c + 1) * 512)
                nc.tensor.matmul(ps, ones[:], src[:, cs],
                                 start=(done[key] == 0),
                                 stop=(done[key] == ns_total - 1))
                done[key] += 1

        # DMA emission: x leads gt by one tile so ACT (the longest chain)
        # never starves during ramp; gt_i still lands well before its
        # consumers (DVE products, PE S-sum) need it.
        xts, gts = [None] * NT, [None] * NT
        slices = []
        off = 0
        for tfd in TILE_SIZES:
            slices.append(slice(off, off + tfd))
            off += tfd

        def issue_x(i):
            xts[i] = xp.tile([P, TILE_SIZES[i]], f32, tag="xt", name=f"xt{i}")
            nc.sync.dma_start(xts[i][:], x_in[:, slices[i]])

        def issue_gt(i):
            gts[i] = gp.tile([P, TILE_SIZES[i]], bf16, tag="gt", name=f"gt{i}")
            nc.sync.dma_start(gts[i][:], g_in[:, slices[i]])

        issue_x(0)
        for i, tfd in enumerate(TILE_SIZES):
            if i + 1 < NT:
                issue_x(i + 1)
            issue_gt(i)
            xt, gt_t = xts[i], gts[i]

            # S += column sums of gt (PE)
            reduce_chunks(psS, "S", gt_t, tfd)

            # pass 1: ln(x); A's fused clamp+mult+reduce goes on DVE
            lnx = lp.tile([P, tfd], bf16, tag="lnx")
            nc.scalar.activation(lnx[:], xt[:], Ln)
            junk = jp.tile([P, tfd], bf16, tag="junk")
            nc.vector.scalar_tensor_tensor(
                junk[:], lnx[:], LOG_CLAMP, gt_t[:],
                AluOpType.max, AluOpType.mult,
                accum_out=accA[:, i : i + 1],
            )

            # pass 2: ln(1-x); B = sum(gt*ln1) via DVE product + PE reduce
            ln1 = lp.tile([P, tfd], bf16, tag="ln1")
            nc.scalar.activation(ln1[:], xt[:], Ln, bias=1.0, scale=-1.0)
            prodB = prp.tile([P, tfd], bf16, tag="prodB")
            nc.vector.tensor_tensor(prodB[:], gt_t[:], ln1[:], AluOpType.mult)
            reduce_chunks(psB, "B", prodB, tfd)
            reduce_chunks(psC, "C", ln1, tfd)

        # drain psums to SBUF: the S bank finishes before B+C, copy in two
        # pieces so the first overlaps the last ln1/products
        sbAll = accp.tile([1, 3 * 512], f32)
        nc.scalar.copy(sbAll[:, 0:512], psAll[:, 0:512])
        nc.scalar.copy(sbAll[:, 512:1536], psAll[:, 512:1536])
        nc.sync.dma_start(outSBC[:], sbAll[:])
        nc.sync.dma_start(outA[:], accA[:])

    nc.compile()
    return nc


def get_nc():
    global _NC_CACHE
    if _NC_CACHE is None:
        _NC_CACHE = _build()
    return _NC_CACHE


def make_in_maps(x, gt):
    x = np.ascontiguousarray(np.asarray(x, dtype=np.float32).reshape(-1))
    gt = np.asarray(gt).reshape(-1)
    assert x.shape == (N_TOTAL,) and gt.shape == (N_TOTAL,)
    # narrow the 0/1 labels to bf16 (exact): 2/3 the DMA bytes, PE-compatible
    gtb = np.ascontiguousarray(gt.astype(ml_dtypes.bfloat16))
    in_maps = []
    for c in range(N_CORES):
        sl = slice(c * PER_CORE, (c + 1) * PER_CORE)
        in_maps.append({
            "x": x[sl].reshape(P, FD),
            "gt": gtb[sl].reshape(P, FD),
        })
    return in_maps


def combine(results):
    """All-reduce the per-core partial sums and finish the loss formula."""
    A = B = C = S = 0.0
    for r in results:
        A += r["outA"].astype(np.float64).sum()
        sbc = r["outSBC"].astype(np.float64).reshape(3, 512)
        S += sbc[0].sum()
        B += sbc[1].sum()
        C += sbc[2].sum()
    n = float(N_TOTAL)
    result = -(A / (2.0 * S) + (C - B) / (2.0 * (n - S)))
    return np.array(result, dtype=np.float32)


def kernel(x, gt):
    global LAST_RESULTS
    nc = get_nc()
    in_maps = make_in_maps(x, gt)
    br = run_bass_kernel_spmd(nc, in_maps, list(range(N_CORES)))
    LAST_RESULTS = br
    return combine(br.results)


# revision 42
# speedup vs baseline: 1.0050x; 1.0050x over previous
"""Weighted-BCE loss kernel for Trainium2 (8 NeuronCores, SPMD data-parallel).

Reference math (torch-style BCELoss with class-balancing weights):
    n   = len(x), s = sum(gt)
    w0  = n / (2*(n-s)),  w1 = n / (2*s)
    L1  = max(log(x),     -100)
    L0  = max(log1p(-x),  -100)
    loss = mean( where(gt==0, w0, w1) * -(gt*L1 + (1-gt)*L0) )

The weights depend only on the GLOBAL positive count s, so the loss
decomposes into 4 global sums computed shard-locally:
    A = sum(gt * L1),  B = sum(gt * L0u),  C = sum(L0u),  s = sum(gt)
    loss = -( A/(2s) + (C-B)/(2(n-s)) )
L0u is UNclamped log(1-x): x is fp32 in [0,1), so 1-x >= 2^-25 and
log(1-x) >= -17.4 — the -100 clamp can never fire on the L0 branch.
The L1 clamp IS needed (x == 0 -> ACT Ln gives -inf, measured) and rides
for free inside the DVE's fused scalar_tensor_tensor op.  The lnx fed to
the PE path is made finite the same way at no cost: Ln(x + 1e-30) via
the ACT affine (error only at x==0: ln(1e-30) vs -100 -> ~2e-6 on the
final loss, vs 2e-2 tolerance).

Schedule (per 1/8 shard, 2M elements as [128 partitions, 16384 free]):
  - gt narrowed to bf16 on the host (0/1 — exact): 2/3 the DMA bytes.
  - DMA: single sync HWDGE ring at ~410 GB/s; x tiles lead gt tiles by
    one wave so ACT (the serial 2-pass engine) never starves — measured
    gapless.  6 tapered tiles.
  - ScalarE: two Ln passes per tile (1x rate at any dtype — the hard
    floor, ~33us), bf16 out, no accumulators.
  - Work balance for the four sums, DVE ~24us / PE ~23us:
      A: tiles 0-2 fused STT on DVE (1x but single-hop);
         tiles 3-4 DVE 2x tensor_tensor product + PE ones-reduce;
         tile 5 STT on DVE.
      B: tiles 0-4 DVE 2x product + PE ones-reduce; tile 5 STT on DVE.
      C: tiles 0-4 PE ones-reduce of ln1; tile 5 tensor_scalar+accum.
      S: all tiles PE ones-reduce of gt.
    The last tile routes through DVE accumulator columns ONLY, so every
    PSUM stream stops at tile 4 — the psum->SBUF copy and its output DMA
    run in the shadow of the last tile instead of serializing after it.
  - Tail: the only end-chain is ln(1-x)[t5] -> two small STT/TS accums
    -> 3KB outD DMA.
Host combines accumulator columns + psum rows from all 8 cores and
finishes the (tiny) all-reduce + final scalar arithmetic in float64.
"""

import numpy as np
import ml_dtypes
from contextlib import ExitStack

import concourse.bass as bass
import concourse.bacc as bacc
import concourse.mybir as mybir
import concourse.tile as tile
from concourse.alu_op_type import AluOpType
from concourse.bass_utils import run_bass_kernel_spmd

N_TOTAL = 16777216
N_CORES = 8
PER_CORE = N_TOTAL // N_CORES   # 2097152
P = 128
FD = PER_CORE // P              # 16384 free elements per partition
TILE_SIZES = [2048, 3072, 4096, 4608, 2048, 512]   # all multiples of 512
assert sum(TILE_SIZES) == FD
NT = len(TILE_SIZES)
LAST = NT - 1
A_STT_TILES = (0, 1, 2)        # A via fused DVE op (single-hop)
A_TT_TILES = (3, 4)            # A via DVE product + PE reduce
LOG_CLAMP = -100.0
LNX_BIAS = 1e-30

# Optional instrumentation knobs for a driver script (harness never sets them).
TRACE = False
LAST_RESULTS = None

_NC_CACHE = None


def _build():
    f32 = mybir.dt.float32
    bf16 = mybir.dt.bfloat16
    Ln = mybir.ActivationFunctionType.Ln

    nc = bacc.Bacc("TRN2")
    x_in = nc.declare_dram_parameter("x", [P, FD], f32, isOutput=False)
    g_in = nc.declare_dram_parameter("gt", [P, FD], bf16, isOutput=False)
    # accumulator columns: A(t0) A(t1) A(t2) A(t5) B(t5) C(t5)
    outD = nc.declare_dram_parameter("outD", [P, 6], f32, isOutput=True)
    # [S | B(t0-4) | C(t0-4) | A(t3-4)] partial rows, 512 each
    outSBCA = nc.declare_dram_parameter("outSBCA", [1, 2048], f32, isOutput=True)

    with tile.TileContext(nc) as tc, ExitStack() as ctx:
        xp = ctx.enter_context(tc.tile_pool(name="xp", bufs=3))
        gp = ctx.enter_context(tc.tile_pool(name="gp", bufs=4))
        lp = ctx.enter_context(tc.tile_pool(name="lp", bufs=2))
        prp = ctx.enter_context(tc.tile_pool(name="prp", bufs=2))
        jp = ctx.enter_context(tc.tile_pool(name="jp", bufs=1))
        accp = ctx.enter_context(tc.tile_pool(name="accp", bufs=1))
        pp = ctx.enter_context(tc.psum_pool(name="pp", bufs=1))

        accD = accp.tile([P, 6], f32)
        ones = accp.tile([P, 1], bf16)
        nc.gpsimd.memset(ones[:], 1.0)
        lnx_bias = accp.tile([P, 1], f32)
        nc.gpsimd.memset(lnx_bias[:], LNX_BIAS)

        # psum banks (2KB/partition each; matmul start-bit resets a bank)
        psAll = pp.tile([1, 4 * 512], f32)
        psS = psAll[:, 0:512]
        psB = psAll[:, 512:1024]
        psC = psAll[:, 1024:1536]
        psA = psAll[:, 1536:2048]

        n_S = FD // 512
        n_BC = (FD - TILE_SIZES[LAST]) // 512
        n_A = sum(TILE_SIZES[i] for i in A_TT_TILES) // 512
        done = {"S": 0, "B": 0, "C": 0, "A": 0}
        total = {"S": n_S, "B": n_BC, "C": n_BC, "A": n_A}

        def reduce_chunks(ps, key, src, tfd):
            for c in range(tfd // 512):
                cs = slice(c * 512, (c + 1) * 512)
                nc.tensor.matmul(ps, ones[:], src[:, cs],
                                 start=(done[key] == 0),
                                 stop=(done[key] == total[key] - 1))
                done[key] += 1

        # DMA emission: x leads gt by one tile (ACT never starves)
        xts, gts = [None] * NT, [None] * NT
        slices = []
        off = 0
        for tfd in TILE_SIZES:
            slices.append(slice(off, off + tfd))
            off += tfd

        def issue_x(i):
            xts[i] = xp.tile([P, TILE_SIZES[i]], f32, tag="xt", name=f"xt{i}")
            nc.sync.dma_start(xts[i][:], x_in[:, slices[i]])

        def issue_gt(i):
            gts[i] = gp.tile([P, TILE_SIZES[i]], bf16, tag="gt", name=f"gt{i}")
            nc.sync.dma_start(gts[i][:], g_in[:, slices[i]])

        issue_x(0)
        for i, tfd in enumerate(TILE_SIZES):
            if i + 1 < NT:
                issue_x(i + 1)
            issue_gt(i)
            xt, gt_t = xts[i], gts[i]

            # S += column sums of gt (PE, all tiles)
            reduce_chunks(psS, "S", gt_t, tfd)

            # pass 1: lnx = ln(x + 1e-30) (finite everywhere)
            lnx = lp.tile([P, tfd], bf16, tag="lnx")
            nc.scalar.activation(lnx[:], xt[:], Ln, bias=lnx_bias[:])
            if i in A_STT_TILES:
                junk = jp.tile([P, tfd], bf16, tag="junk")
                nc.vector.scalar_tensor_tensor(
                    junk[:], lnx[:], LOG_CLAMP, gt_t[:],
                    AluOpType.max, AluOpType.mult,
                    accum_out=accD[:, i : i + 1],
                )
            elif i in A_TT_TILES:
                prodA = prp.tile([P, tfd], bf16, tag="prodA")
                nc.vector.tensor_tensor(prodA[:], gt_t[:], lnx[:],
                                        AluOpType.mult)
                reduce_chunks(psA, "A", prodA, tfd)
            else:  # last tile: fused accum, keeps PE stops at tile 4
                junk = jp.tile([P, tfd], bf16, tag="junk")
                nc.vector.scalar_tensor_tensor(
                    junk[:], lnx[:], LOG_CLAMP, gt_t[:],
                    AluOpType.max, AluOpType.mult,
                    accum_out=accD[:, 3:4],
                )

            # pass 2: ln1 = ln(1-x)
            ln1 = lp.tile([P, tfd], bf16, tag="ln1")
            nc.scalar.activation(ln1[:], xt[:], Ln, bias=1.0, scale=-1.0)
            if i != LAST:
                prodB = prp.tile([P, tfd], bf16, tag="prodB")
                nc.vector.tensor_tensor(prodB[:], gt_t[:], ln1[:],
                                        AluOpType.mult)
                reduce_chunks(psB, "B", prodB, tfd)
                reduce_chunks(psC, "C", ln1, tfd)
            else:
                junkB = jp.tile([P, tfd], bf16, tag="junk")
                nc.vector.scalar_tensor_tensor(
                    junkB[:], ln1[:], LOG_CLAMP, gt_t[:],
                    AluOpType.max, AluOpType.mult,
                    accum_out=accD[:, 4:5],
                )
                junkC = jp.tile([P, tfd], bf16, tag="junk")
                nc.vector.tensor_scalar(
                    junkC[:], ln1[:], 1.0, 0.0, AluOpType.mult, AluOpType.add,
                    accum_out=accD[:, 5:6],
                )

        # psum streams all stopped at tile 4: copy + DMA overlap tile 5
        sbAll = accp.tile([1, 4 * 512], f32)
        nc.scalar.copy(sbAll[:], psAll[:])
        nc.sync.dma_start(outSBCA[:], sbAll[:])
        nc.sync.dma_start(outD[:], accD[:])

    nc.compile()
    return nc


def get_nc():
    global _NC_CACHE
    if _NC_CACHE is None:
        _NC_CACHE = _build()
    return _NC_CACHE


def make_in_maps(x, gt):
    x = np.ascontiguousarray(np.asarray(x, dtype=np.float32).reshape(-1))
    gt = np.asarray(gt).reshape(-1)
    assert x.shape == (N_TOTAL,) and gt.shape == (N_TOTAL,)
    # narrow the 0/1 labels to bf16 (exact): 2/3 the DMA bytes, PE-compatible
    gtb = np.ascontiguousarray(gt.astype(ml_dtypes.bfloat16))
    in_maps = []
    for c in range(N_CORES):
        sl = slice(c * PER_CORE, (c + 1) * PER_CORE)
        in_maps.append({
            "x": x[sl].reshape(P, FD),
            "gt": gtb[sl].reshape(P, FD),
        })
    return in_maps


def combine(results):
    """All-reduce the per-core partial sums and finish the loss formula."""
    A = B = C = S = 0.0
    for r in results:
        d = r["outD"].astype(np.float64)
        sbca = r["outSBCA"].astype(np.float64).reshape(4, 512)
        S += sbca[0].sum()
        B += sbca[1].sum() + d[:, 4].sum()
        C += sbca[2].sum() + d[:, 5].sum()
        A += sbca[3].sum() + d[:, 0:4].sum()
    n = float(N_TOTAL)
    result = -(A / (2.0 * S) + (C - B) / (2.0 * (n - S)))
    return np.array(result, dtype=np.float32)


def kernel(x, gt):
    global LAST_RESULTS
    nc = get_nc()
    in_maps = make_in_maps(x, gt)
    br = run_bass_kernel_spmd(nc, in_maps, list(range(N_CORES)))
    LAST_RESULTS = br
    return combine(br.results)
